# revision 11
# baseline (speedup 1.0000x reference)
"""MoE GroupedExperts kernel for 8 TRN2 NeuronCores.

Expert-parallel: expert e's tokens + weights go to core e. Tokens are
pre-sorted by expert, so routing is host-side slicing. Each core runs a
SwiGLU MLP: o = (silu(x @ gate) * (x @ up)) @ down.

Schedule: the hid dimension is processed in groups of PAIR*128 slices.
Per group g the PE runs gate(g), up(g), then down(g-1) matmuls, while
ACT/DVE produce h(g) = silu(gate_psum)*up_psum in the shadow of the
down/gate streams. The down-proj accumulates into 4 persistent PSUM
banks (tok x dim-chunk) across all groups. Weight DMAs are emitted on
one ring in exact consumption order (gw g, uw g, dw g-1, ...) so the
FIFO ring paces arrival to demand. The host packs weights slice-major
so every DMA (even the per-slice first ones) reads >=2KB contiguous
lines per partition -- small-line DMAs clog the shared SDMA engines.

Device compute in fp16 (fp32 accumulation in PSUM): same PE rate as
bf16 with 3 more mantissa bits.
"""

import sys

if "/opt/trn_rl_repo" not in sys.path:
    sys.path.insert(0, "/opt/trn_rl_repo")

import numpy as np

F16 = np.float16
E = 8
DIM = 1024
HID = 2048
N_CORES = 8
CMAX_BLOCK = 512  # max tokens per device invocation (PSUM free-dim limit)

_cache = {}


def _build(cpad: int):
    """Build + compile the per-core kernel for cpad tokens per expert."""
    from concourse import bacc
    import concourse.tile as tile
    import concourse.mybir as mybir

    f32 = mybir.dt.float32
    f16 = mybir.dt.float16

    KC = DIM // 128       # k-chunks for gate/up contraction
    NH = HID // 128       # hid slices
    PAIR = max(1, min(NH, 512 // cpad))  # hid slices per PSUM bank
    NG = NH // PAIR       # hid groups
    NTOK = cpad // 128    # token tiles
    NDC = DIM // 512      # output dim chunks

    nc = bacc.Bacc("TRN2", target_bir_lowering=False, debug=False)
    # Host-packed layouts: gw/uw are slice-major [s, p, k, h] so any
    # slice run is contiguous (2KB lines per partition); dw group-major.
    xt_d = nc.dram_tensor("xt", [128, KC, cpad], f16, kind="ExternalInput")
    gw_d = nc.dram_tensor("gw", [NH, 128, KC, 128], f16, kind="ExternalInput")
    uw_d = nc.dram_tensor("uw", [NH, 128, KC, 128], f16, kind="ExternalInput")
    dw_d = nc.dram_tensor("dw", [NG, 128, PAIR, DIM], f16, kind="ExternalInput")
    o_d = nc.dram_tensor("o", [cpad, DIM], f16, kind="ExternalOutput")

    # [s, p, k, h] -> [p, s, k, h] views so DMA iteration order matches
    # the SBUF tile layout [p, j, k, h].
    gw_v = gw_d.ap().rearrange("s p k h -> p s k h")
    uw_v = uw_d.ap().rearrange("s p k h -> p s k h")

    with tile.TileContext(nc) as tc:
        with (
            tc.tile_pool(name="sb", bufs=1) as sb,
            tc.tile_pool(name="gwp", bufs=NG) as gwp,
            tc.tile_pool(name="uwp", bufs=NG) as uwp,
            tc.tile_pool(name="dwp", bufs=NG) as dwp,
            tc.tile_pool(name="outp", bufs=NTOK * NDC) as outp,
            tc.tile_pool(name="stmp", bufs=2) as stmp_pool,
            tc.tile_pool(name="ht", bufs=3) as ht_pool,
            tc.tile_pool(name="psA", bufs=2, space="PSUM") as psA,
            tc.tile_pool(name="psB", bufs=2, space="PSUM") as psB,
            tc.tile_pool(name="psO", bufs=NTOK * NDC, space="PSUM") as psO,
        ):
            xt_s = sb.tile([128, KC, cpad], f16)

            # xt on the scalar ring (parallel to the sync weight ring),
            # smallest chunk first so gate MM k=0 can start ASAP.
            xt_v = xt_d.ap()
            q1, q2 = KC // 4, KC // 2
            nc.scalar.dma_start(xt_s[:, :q1, :], xt_v[:, :q1, :])
            nc.scalar.dma_start(xt_s[:, q1:q2, :], xt_v[:, q1:q2, :])
            nc.scalar.dma_start(xt_s[:, q2:, :], xt_v[:, q2:, :])



            # Weight DMAs on the sync ring in consumption order. Group 0
            # is sent one slice at a time so the PE can start ASAP; every
            # transfer keeps >=2KB contiguous lines per partition.
            gw_t, uw_t, dw_t = [], [], []
            for g in range(NG):
                gt = gwp.tile([128, PAIR, KC, 128], f16, tag="gw", name=f"gw{g}")
                ut = uwp.tile([128, PAIR, KC, 128], f16, tag="uw", name=f"uw{g}")
                dt = dwp.tile([128, PAIR, DIM], f16, tag="dw", name=f"dw{g}")
                gw_t.append(gt)
                uw_t.append(ut)
                dw_t.append(dt)
                s0 = g * PAIR
                if g == 0:
                    # Arrival order == MM consumption order: all gate
                    # slices first (k-split so MM k=0 starts earliest),
                    # then all up slices.
                    kh = KC // 2
                    nc.sync.dma_start(
                        gt[:, 0:1, :kh, :], gw_v[:, s0:s0 + 1, :kh, :])
                    nc.sync.dma_start(
                        gt[:, 0:1, kh:, :], gw_v[:, s0:s0 + 1, kh:, :])
                    for j in range(1, PAIR):
                        nc.sync.dma_start(
                            gt[:, j:j + 1, :, :], gw_v[:, s0 + j:s0 + j + 1, :, :])
                    for j in range(PAIR):
                        nc.sync.dma_start(
                            ut[:, j:j + 1, :, :], uw_v[:, s0 + j:s0 + j + 1, :, :])
                else:
                    nc.sync.dma_start(gt[:], gw_v[:, s0:s0 + PAIR, :, :])
                    nc.sync.dma_start(ut[:], uw_v[:, s0:s0 + PAIR, :, :])
                if g >= 1:
                    nc.sync.dma_start(dw_t[g - 1][:], dw_d.ap()[g - 1])
            nc.sync.dma_start(dw_t[NG - 1][:], dw_d.ap()[NG - 1])

            # Persistent down-proj accumulators: tok x dim-chunk PSUM banks.
            po = [
                [psO.tile([128, 512], f32, tag="po", name=f"po{t}_{dc}")
                 for dc in range(NDC)]
                for t in range(NTOK)
            ]

            def down_mms(g, emit_out):
                for t in range(NTOK):
                    t0, t1 = t * 128, (t + 1) * 128
                    for dc in range(NDC):
                        d0, d1 = dc * 512, (dc + 1) * 512
                        for j in range(PAIR):
                            nc.tensor.matmul(
                                po[t][dc][:],
                                ht_t_list[g][:, j, t0:t1],
                                dw_t[g][:, j, d0:d1],
                                start=(g == 0 and j == 0),
                                stop=(g == NG - 1 and j == PAIR - 1),
                                skip_group_check=True,
                            )
                        if emit_out:
                            # Drain each finished accumulator immediately;
                            # alternate rings so the two final output DMAs
                            # overlap.
                            out_s = outp.tile(
                                [128, 512], f16, tag="out", name=f"out{t}_{dc}")
                            nc.vector.tensor_copy(out_s[:], po[t][dc][:])
                            eng = nc.scalar if (t * NDC + dc) % 2 == 0 else nc.sync
                            eng.dma_start(o_d[t0:t1, d0:d1], out_s[:])

            ht_t_list = []
            for g in range(NG):
                pg = psA.tile([128, PAIR, cpad], f32, tag="pg")
                pu = psB.tile([128, PAIR, cpad], f32, tag="pu")
                for j in range(PAIR):
                    for k in range(KC):
                        nc.tensor.matmul(
                            pg[:, j, :], gw_t[g][:, j, k, :], xt_s[:, k, :],
                            start=(k == 0), stop=(k == KC - 1),
                            skip_group_check=True,
                        )
                for j in range(PAIR):
                    for k in range(KC):
                        nc.tensor.matmul(
                            pu[:, j, :], uw_t[g][:, j, k, :], xt_s[:, k, :],
                            start=(k == 0), stop=(k == KC - 1),
                            skip_group_check=True,
                        )
                stmp = stmp_pool.tile([128, PAIR, cpad], f32, tag="stmp")
                nc.scalar.activation(
                    stmp[:], pg[:], mybir.ActivationFunctionType.Silu
                )
                ht_t = ht_pool.tile([128, PAIR, cpad], f16, tag="ht")
                nc.vector.tensor_mul(ht_t[:], stmp[:], pu[:])
                ht_t_list.append(ht_t)
                if g >= 1:
                    down_mms(g - 1, emit_out=False)
            down_mms(NG - 1, emit_out=True)

    nc.compile()
    return nc


def _get_nc(cpad: int):
    if cpad not in _cache:
        _cache[cpad] = _build(cpad)
    return _cache[cpad]


def _pack_weights(gate_proj, up_proj, down_proj, cpad: int):
    """Per-expert packed DRAM layouts matching _build's dram tensors."""
    KC = DIM // 128
    NH = HID // 128
    PAIR = max(1, min(NH, 512 // cpad))
    NG = NH // PAIR
    packed = []
    for e in range(E):
        g16 = gate_proj[e].astype(F16)
        u16 = up_proj[e].astype(F16)
        d16 = down_proj[e].astype(F16)
        # gw[d, h] -> [s, p, k, hh] with d = k*128+p, h = s*128+hh
        gp = np.ascontiguousarray(
            g16.reshape(KC, 128, NH, 128).transpose(2, 1, 0, 3))
        upk = np.ascontiguousarray(
            u16.reshape(KC, 128, NH, 128).transpose(2, 1, 0, 3))
        # dw[hk, d] -> [g, p, j, d] with hk = (g*PAIR+j)*128+p
        dp = np.ascontiguousarray(
            d16.reshape(NG, PAIR, 128, DIM).transpose(0, 2, 1, 3))
        packed.append((gp, upk, dp))
    return packed


def _run_block(nc, xt_blocks, packed, collect):
    """One SPMD invocation: xt_blocks[e] is packed [128, KC, cpad] f16."""
    from concourse.bass_utils import run_bass_kernel_spmd

    in_maps = []
    for e in range(E):
        gp, upk, dp = packed[e]
        in_maps.append({"xt": xt_blocks[e], "gw": gp, "uw": upk, "dw": dp})
    kwargs = {} if collect is None else dict(collect.get("run_kwargs") or {})
    res = run_bass_kernel_spmd(nc, in_maps, core_ids=list(range(N_CORES)), **kwargs)
    if collect is not None:
        collect.setdefault("results", []).append(res)
    return [res.results[e]["o"] for e in range(E)]


def kernel(x, counts, gate_proj, up_proj, down_proj, _collect=None):
    x = np.ascontiguousarray(np.asarray(x, dtype=np.float32))
    counts = np.asarray(counts, dtype=np.int32)
    gate_proj = np.asarray(gate_proj, dtype=np.float32)
    up_proj = np.asarray(up_proj, dtype=np.float32)
    down_proj = np.asarray(down_proj, dtype=np.float32)

    T = x.shape[0]
    offs = np.concatenate([[0], np.cumsum(counts)]).astype(np.int64)
    cmax = int(counts.max()) if counts.size else 128

    n_blocks = max(1, -(-cmax // CMAX_BLOCK))
    if n_blocks == 1:
        cpad = max(128, -(-cmax // 128) * 128)
    else:
        cpad = CMAX_BLOCK

    KC = DIM // 128
    nc = _get_nc(cpad)
    packed = _pack_weights(gate_proj, up_proj, down_proj, cpad)

    out = np.empty((T, DIM), dtype=np.float32)  # o arrives fp16, upcast here
    for b in range(n_blocks):
        xt_blocks = []
        spans = []
        for e in range(E):
            c = int(counts[e])
            s0 = min(b * cpad, c)
            s1 = min((b + 1) * cpad, c)
            xe = x[offs[e] + s0:offs[e] + s1]
            if xe.shape[0] < cpad:
                xe = np.concatenate(
                    [xe, np.zeros((cpad - xe.shape[0], DIM), np.float32)], axis=0
                )
            # x[c, d] -> xt[p, k, c] with d = k*128+p
            xt = np.ascontiguousarray(
                xe.astype(F16).T.reshape(KC, 128, cpad).transpose(1, 0, 2))
            xt_blocks.append(xt)
            spans.append((s0, s1))
        outs = _run_block(nc, xt_blocks, packed, _collect)
        for e in range(E):
            s0, s1 = spans[e]
            if s1 > s0:
                out[offs[e] + s0:offs[e] + s1] = outs[e][: s1 - s0]
    return out


# revision 13
# speedup vs baseline: 1.0115x; 1.0115x over previous
"""MoE GroupedExperts kernel for 8 TRN2 NeuronCores.

Expert-parallel: expert e's tokens + weights go to core e. Tokens are
pre-sorted by expert, so routing is host-side slicing. Each core runs a
SwiGLU MLP: o = (silu(x @ gate) * (x @ up)) @ down.

Schedule: the hid dimension is processed in groups of PAIR*128 slices.
Per group g the PE runs gate(g), up(g), then down(g-1) matmuls, while
ACT/DVE produce h(g) = silu(gate_psum)*up_psum in the shadow of the
down/gate streams. The down-proj accumulates into 4 persistent PSUM
banks (tok x dim-chunk) across all groups. Weight DMAs are emitted on
one ring in exact consumption order (gw g, uw g, dw g-1, ...) so the
FIFO ring paces arrival to demand. The host packs weights slice-major
so every DMA (even the per-slice first ones) reads >=2KB contiguous
lines per partition -- small-line DMAs clog the shared SDMA engines.

Device compute in fp16 (fp32 accumulation in PSUM): same PE rate as
bf16 with 3 more mantissa bits.
"""

import sys

if "/opt/trn_rl_repo" not in sys.path:
    sys.path.insert(0, "/opt/trn_rl_repo")

import numpy as np

F16 = np.float16
E = 8
DIM = 1024
HID = 2048
N_CORES = 8
CMAX_BLOCK = 512  # max tokens per device invocation (PSUM free-dim limit)

_cache = {}


def _build(cpad: int):
    """Build + compile the per-core kernel for cpad tokens per expert."""
    from concourse import bacc
    import concourse.tile as tile
    import concourse.mybir as mybir

    f32 = mybir.dt.float32
    f16 = mybir.dt.float16

    KC = DIM // 128       # k-chunks for gate/up contraction
    NH = HID // 128       # hid slices
    PAIR = max(1, min(NH, 512 // cpad))  # hid slices per PSUM bank
    NG = NH // PAIR       # hid groups
    NTOK = cpad // 128    # token tiles
    NDC = DIM // 512      # output dim chunks

    nc = bacc.Bacc("TRN2", target_bir_lowering=False, debug=False)
    # Host-packed layouts: gw/uw are slice-major [s, p, k, h] so any
    # slice run is contiguous (2KB lines per partition); dw group-major.
    xt_d = nc.dram_tensor("xt", [128, KC, cpad], f16, kind="ExternalInput")
    gw_d = nc.dram_tensor("gw", [NH, 128, KC, 128], f16, kind="ExternalInput")
    uw_d = nc.dram_tensor("uw", [NH, 128, KC, 128], f16, kind="ExternalInput")
    dw_d = nc.dram_tensor("dw", [NG, 128, PAIR, DIM], f16, kind="ExternalInput")
    o_d = nc.dram_tensor("o", [cpad, DIM], f16, kind="ExternalOutput")

    # [s, p, k, h] -> [p, s, k, h] views so DMA iteration order matches
    # the SBUF tile layout [p, j, k, h].
    gw_v = gw_d.ap().rearrange("s p k h -> p s k h")
    uw_v = uw_d.ap().rearrange("s p k h -> p s k h")

    with tile.TileContext(nc) as tc:
        with (
            tc.tile_pool(name="sb", bufs=1) as sb,
            tc.tile_pool(name="gwp", bufs=NG) as gwp,
            tc.tile_pool(name="uwp", bufs=NG) as uwp,
            tc.tile_pool(name="dwp", bufs=NG) as dwp,
            tc.tile_pool(name="outp", bufs=NTOK * NDC) as outp,
            tc.tile_pool(name="stmp", bufs=2) as stmp_pool,
            tc.tile_pool(name="ht", bufs=3) as ht_pool,
            tc.tile_pool(name="psA", bufs=2, space="PSUM") as psA,
            tc.tile_pool(name="psB", bufs=2, space="PSUM") as psB,
            tc.tile_pool(name="psO", bufs=NTOK * NDC, space="PSUM") as psO,
        ):
            xt_s = sb.tile([128, KC, cpad], f16)

            # xt on the scalar ring (parallel to the sync weight ring), in
            # two halves so gate MMs k=0..KC/2-1 can start after the first.
            xt_v = xt_d.ap()
            h0 = KC // 2
            nc.scalar.dma_start(xt_s[:, :h0, :], xt_v[:, :h0, :])
            nc.scalar.dma_start(xt_s[:, h0:, :], xt_v[:, h0:, :])



            # Weight DMAs on the sync ring in consumption order. Group 0
            # is sent one slice at a time so the PE can start ASAP; every
            # transfer keeps >=2KB contiguous lines per partition.
            gw_t, uw_t, dw_t = [], [], []
            for g in range(NG):
                gt = gwp.tile([128, PAIR, KC, 128], f16, tag="gw", name=f"gw{g}")
                ut = uwp.tile([128, PAIR, KC, 128], f16, tag="uw", name=f"uw{g}")
                dt = dwp.tile([128, PAIR, DIM], f16, tag="dw", name=f"dw{g}")
                gw_t.append(gt)
                uw_t.append(ut)
                dw_t.append(dt)
                s0 = g * PAIR
                nc.sync.dma_start(gt[:], gw_v[:, s0:s0 + PAIR, :, :])
                nc.sync.dma_start(ut[:], uw_v[:, s0:s0 + PAIR, :, :])
                if g >= 1:
                    nc.sync.dma_start(dw_t[g - 1][:], dw_d.ap()[g - 1])
            nc.sync.dma_start(dw_t[NG - 1][:], dw_d.ap()[NG - 1])

            # Persistent down-proj accumulators: tok x dim-chunk PSUM banks.
            po = [
                [psO.tile([128, 512], f32, tag="po", name=f"po{t}_{dc}")
                 for dc in range(NDC)]
                for t in range(NTOK)
            ]

            def down_mms(g, emit_out):
                for t in range(NTOK):
                    t0, t1 = t * 128, (t + 1) * 128
                    for dc in range(NDC):
                        d0, d1 = dc * 512, (dc + 1) * 512
                        for j in range(PAIR):
                            nc.tensor.matmul(
                                po[t][dc][:],
                                ht_t_list[g][:, j, t0:t1],
                                dw_t[g][:, j, d0:d1],
                                start=(g == 0 and j == 0),
                                stop=(g == NG - 1 and j == PAIR - 1),
                                skip_group_check=True,
                            )
                        if emit_out:
                            # Drain each finished accumulator immediately;
                            # alternate rings so the two final output DMAs
                            # overlap.
                            out_s = outp.tile(
                                [128, 512], f16, tag="out", name=f"out{t}_{dc}")
                            nc.vector.tensor_copy(out_s[:], po[t][dc][:])
                            eng = nc.scalar if (t * NDC + dc) % 2 == 0 else nc.sync
                            eng.dma_start(o_d[t0:t1, d0:d1], out_s[:])

            ht_t_list = []
            for g in range(NG):
                pg = psA.tile([128, PAIR, cpad], f32, tag="pg")
                pu = psB.tile([128, PAIR, cpad], f32, tag="pu")
                for j in range(PAIR):
                    for k in range(KC):
                        nc.tensor.matmul(
                            pg[:, j, :], gw_t[g][:, j, k, :], xt_s[:, k, :],
                            start=(k == 0), stop=(k == KC - 1),
                            skip_group_check=True,
                        )
                for j in range(PAIR):
                    for k in range(KC):
                        nc.tensor.matmul(
                            pu[:, j, :], uw_t[g][:, j, k, :], xt_s[:, k, :],
                            start=(k == 0), stop=(k == KC - 1),
                            skip_group_check=True,
                        )
                stmp = stmp_pool.tile([128, PAIR, cpad], f32, tag="stmp")
                nc.scalar.activation(
                    stmp[:], pg[:], mybir.ActivationFunctionType.Silu
                )
                ht_t = ht_pool.tile([128, PAIR, cpad], f16, tag="ht")
                nc.vector.tensor_mul(ht_t[:], stmp[:], pu[:])
                ht_t_list.append(ht_t)
                if g >= 1:
                    down_mms(g - 1, emit_out=False)
            down_mms(NG - 1, emit_out=True)

    nc.compile()
    return nc


def _get_nc(cpad: int):
    if cpad not in _cache:
        _cache[cpad] = _build(cpad)
    return _cache[cpad]


def _pack_weights(gate_proj, up_proj, down_proj, cpad: int):
    """Per-expert packed DRAM layouts matching _build's dram tensors."""
    KC = DIM // 128
    NH = HID // 128
    PAIR = max(1, min(NH, 512 // cpad))
    NG = NH // PAIR
    packed = []
    for e in range(E):
        g16 = gate_proj[e].astype(F16)
        u16 = up_proj[e].astype(F16)
        d16 = down_proj[e].astype(F16)
        # gw[d, h] -> [s, p, k, hh] with d = k*128+p, h = s*128+hh
        gp = np.ascontiguousarray(
            g16.reshape(KC, 128, NH, 128).transpose(2, 1, 0, 3))
        upk = np.ascontiguousarray(
            u16.reshape(KC, 128, NH, 128).transpose(2, 1, 0, 3))
        # dw[hk, d] -> [g, p, j, d] with hk = (g*PAIR+j)*128+p
        dp = np.ascontiguousarray(
            d16.reshape(NG, PAIR, 128, DIM).transpose(0, 2, 1, 3))
        packed.append((gp, upk, dp))
    return packed


def _run_block(nc, xt_blocks, packed, collect):
    """One SPMD invocation: xt_blocks[e] is packed [128, KC, cpad] f16."""
    from concourse.bass_utils import run_bass_kernel_spmd

    in_maps = []
    for e in range(E):
        gp, upk, dp = packed[e]
        in_maps.append({"xt": xt_blocks[e], "gw": gp, "uw": upk, "dw": dp})
    kwargs = {} if collect is None else dict(collect.get("run_kwargs") or {})
    res = run_bass_kernel_spmd(nc, in_maps, core_ids=list(range(N_CORES)), **kwargs)
    if collect is not None:
        collect.setdefault("results", []).append(res)
    return [res.results[e]["o"] for e in range(E)]


def kernel(x, counts, gate_proj, up_proj, down_proj, _collect=None):
    x = np.ascontiguousarray(np.asarray(x, dtype=np.float32))
    counts = np.asarray(counts, dtype=np.int32)
    gate_proj = np.asarray(gate_proj, dtype=np.float32)
    up_proj = np.asarray(up_proj, dtype=np.float32)
    down_proj = np.asarray(down_proj, dtype=np.float32)

    T = x.shape[0]
    offs = np.concatenate([[0], np.cumsum(counts)]).astype(np.int64)
    cmax = int(counts.max()) if counts.size else 128

    n_blocks = max(1, -(-cmax // CMAX_BLOCK))
    if n_blocks == 1:
        cpad = max(128, -(-cmax // 128) * 128)
    else:
        cpad = CMAX_BLOCK

    KC = DIM // 128
    nc = _get_nc(cpad)
    packed = _pack_weights(gate_proj, up_proj, down_proj, cpad)

    out = np.empty((T, DIM), dtype=np.float32)  # o arrives fp16, upcast here
    for b in range(n_blocks):
        xt_blocks = []
        spans = []
        for e in range(E):
            c = int(counts[e])
            s0 = min(b * cpad, c)
            s1 = min((b + 1) * cpad, c)
            xe = x[offs[e] + s0:offs[e] + s1]
            if xe.shape[0] < cpad:
                xe = np.concatenate(
                    [xe, np.zeros((cpad - xe.shape[0], DIM), np.float32)], axis=0
                )
            # x[c, d] -> xt[p, k, c] with d = k*128+p
            xt = np.ascontiguousarray(
                xe.astype(F16).T.reshape(KC, 128, cpad).transpose(1, 0, 2))
            xt_blocks.append(xt)
            spans.append((s0, s1))
        outs = _run_block(nc, xt_blocks, packed, _collect)
        for e in range(E):
            s0, s1 = spans[e]
            if s1 > s0:
                out[offs[e] + s0:offs[e] + s1] = outs[e][: s1 - s0]
    return out
